# revision 1
# baseline (speedup 1.0000x reference)
"""Trainium2 Bass kernel for nn_DS_Attention_7636451852327.

Data-parallel over batch: 32 batches -> 8 NeuronCores, 4 batches (2048 tokens)
per core. Per core, per 128-token tile:
  DMA q/v fp32 -> cast fp16 -> PE-transpose -> X^T tiles
  QKV matmul (lhsT = X^T, rhs = host-permuted weights) -> token-major qa/ka/va (fp16)
  per-frame 24x24(x6) attention via broadcast-AP DVE ops + ACT exp, with the
  custom weighting restructured as:
    row-0 chain applied on unnormalized exp-scores (scale-invariant),
    col-0 chain on u = e[:,0]*recip(rowsum), fold normalization into the
    attention output, plus a du * va[0] rank-1 correction.
  assemble v'_tok -> PE-transpose -> final matmul + bias -> fp32 out.
"""
import numpy as np
import ml_dtypes
from contextlib import ExitStack

import concourse.bass as bass
import concourse.mybir as mybir
import concourse.tile as tile
from concourse import bacc
from concourse.bass_utils import run_bass_kernel_spmd
from concourse.masks import make_identity

import os as _os0
# fp16 gives ~8x better accuracy than bf16 at identical speed (set LP_BF16=1 to revert)
bf = mybir.dt.bfloat16 if _os0.environ.get("LP_BF16") else mybir.dt.float16
f32 = mybir.dt.float32
AL = mybir.AluOpType
AX = mybir.AxisListType
AF = mybir.ActivationFunctionType

P = 128
H = 8
QJ = KJ = 24
D = 6
NQK = QJ * D              # 144
NVA = H * NQK             # 1152
DH = 1176                 # 147*8
D_MODEL = 512
W_TOT = 3 * NVA + H * 3   # 3480
B = 32
N = 512
N_CORES = 8
TT = (B // N_CORES) * N // P   # 16 token tiles per core
CHAIN = [(6, 3), (9, 6), (12, 9), (13, 9), (14, 9), (16, 13), (17, 14), (15, 12)]
import os as _os
# HW-verified default config: 2x-mode add-trees for all reductions, plain
# TT+TS pairs for the weighting chains. tensor_tensor_reduce (TTR) chain ops
# fault TRN2 hardware (NRT_EXEC_UNIT_UNRECOVERABLE) despite passing CoreSim —
# keep V1_CHAINS unless TTR_CHAINS=1.
V1_DSUM = bool(_os.environ.get("V1_DSUM"))
V1_ROWSUM = bool(_os.environ.get("V1_ROWSUM"))
V1_CHAINS = not _os.environ.get("TTR_CHAINS")
V1_KJSUM = bool(_os.environ.get("V1_KJSUM"))

QKV_CHUNKS = [
    (0, 512, 0), (512, 1024, 0), (1024, 1152, 0),
    (1152, 1664, 1), (1664, 2176, 1), (2176, 2304, 1),
    (2304, 2816, 2), (2816, 3328, 2), (3328, 3480, 2),
]


def _inner_attention(tc, work, small, qa_all, ka_all, va_all, vptok):
    nc = tc.nc
    NP = QJ * KJ  # 576 pairs
    e_all = work.tile([P, H * QJ * KJ], bf, tag="e_all")
    for h in range(H):
        qa = qa_all[:, h * NQK:(h + 1) * NQK]
        ka = ka_all[:, h * NQK:(h + 1) * NQK]
        p1 = work.tile([P, QJ * KJ * D], bf, tag="p1")
        p1v = p1[:].rearrange("p (q k d) -> p q k d", q=QJ, k=KJ)
        qa_b = qa.rearrange("p (q d) -> p q d", q=QJ).unsqueeze(2).broadcast_to([P, QJ, KJ, D])
        ka_b = ka.rearrange("p (k d) -> p k d", k=KJ).unsqueeze(1).broadcast_to([P, QJ, KJ, D])
        nc.vector.tensor_tensor(p1v, qa_b, ka_b, AL.mult)
        if V1_DSUM:
            sc = work.tile([P, NP], bf, tag="sc")
            with nc.allow_low_precision(reason="bf16 scores"):
                nc.vector.tensor_reduce(
                    sc[:].rearrange("p (q k) -> p q k", q=QJ), p1v, axis=AX.X, op=AL.add)
            nc.scalar.activation(e_all[:, h * NP:(h + 1) * NP], sc[:], AF.Exp)
        else:
            # d-sum as 2x-mode adds of [pair, 2] slices: s2 = p1[0:2] + p1[2:4] + p1[4:6]
            p1p = p1[:].rearrange("p (pr d) -> p pr d", pr=NP)
            s2a = work.tile([P, NP * 2], bf, tag="s2a")
            s2av = s2a[:].rearrange("p (pr d) -> p pr d", pr=NP)
            nc.vector.tensor_tensor(s2av, p1p[:, :, 0:2], p1p[:, :, 2:4], AL.add)
            nc.vector.tensor_tensor(s2av, s2av, p1p[:, :, 4:6], AL.add)
            # exp(a+b) = exp(a)*exp(b): two strided exps on ACT, one 2x mul on DVE
            ea = work.tile([P, NP], bf, tag="ea")
            eb = work.tile([P, NP], bf, tag="eb")
            nc.scalar.activation(ea[:], s2av[:, :, 0], AF.Exp)
            nc.scalar.activation(eb[:], s2av[:, :, 1], AF.Exp)
            nc.vector.tensor_tensor(e_all[:, h * NP:(h + 1) * NP], ea[:], eb[:], AL.mult)
    eav = e_all[:].rearrange("p (h q k) -> p h q k", h=H, q=QJ)
    s_all = small.tile([P, H * QJ], f32, tag="s_all")
    if V1_ROWSUM:
        nc.vector.tensor_reduce(
            s_all[:].rearrange("p (h q) -> p h q", h=H), eav, axis=AX.X, op=AL.add)
    else:
        # rowsum over kj via 2x-mode halving adds: 24 -> 12 -> 6 -> 2 -> 1
        HQ = H * QJ
        eflat = e_all[:].rearrange("p (hq k) -> p hq k", hq=HQ)
        r12 = small.tile([P, HQ * 12], bf, tag="r12")
        r12v = r12[:].rearrange("p (f k) -> p f k", f=HQ)
        nc.vector.tensor_tensor(r12v, eflat[:, :, 0:12], eflat[:, :, 12:24], AL.add)
        r6 = small.tile([P, HQ * 6], bf, tag="r6")
        r6v = r6[:].rearrange("p (f k) -> p f k", f=HQ)
        nc.vector.tensor_tensor(r6v, r12v[:, :, 0:6], r12v[:, :, 6:12], AL.add)
        r2t = small.tile([P, HQ * 2], bf, tag="r2t")
        r2v = r2t[:].rearrange("p (f k) -> p f k", f=HQ)
        nc.vector.tensor_tensor(r2v, r6v[:, :, 0:2], r6v[:, :, 2:4], AL.add)
        nc.vector.tensor_tensor(r2v, r2v, r6v[:, :, 4:6], AL.add)
        nc.vector.tensor_tensor(s_all[:], r2v[:, :, 0], r2v[:, :, 1], AL.add)
    r_all = small.tile([P, H * QJ], f32, tag="r_all")
    nc.vector.reciprocal(r_all[:], s_all[:])
    u_all = small.tile([P, H * QJ], f32, tag="u_all")
    rav = r_all[:].rearrange("p (h q) -> p h q", h=H)
    nc.vector.tensor_tensor(
        u_all[:].rearrange("p (h q) -> p h q", h=H), eav[:, :, :, 0], rav, AL.mult)
    u_orig = small.tile([P, H * QJ], f32, tag="u_orig")
    nc.scalar.copy(u_orig[:], u_all[:])
    uav = u_all[:].rearrange("p (h q) -> p h q", h=H)
    if V1_CHAINS:
        # Leveled chain: the sequential avg-chain is a 4-level DAG; levels 3/4
        # hit contiguous column runs (12:15 <- 9, 15:18 <- 12:15), so each
        # level is one strided TT add + one TS halving over all its targets.
        tmp8 = small.tile([P, H * 3], bf, tag="tmp8")
        tmp8f = small.tile([P, H * 3], f32, tag="tmp8f")
        for view, tmp in ((eav[:, :, 0, :], tmp8), (uav, tmp8f)):
            t3 = tmp[:].rearrange("p (h c) -> p h c", h=H)
            for dsl, ssl in (((6, 7), (3, 4)), ((9, 10), (6, 7)),
                             ((12, 15), (9, 10)), ((15, 18), (12, 15))):
                nd = dsl[1] - dsl[0]
                dst = view[:, :, dsl[0]:dsl[1]]
                src = view[:, :, ssl[0]:ssl[1]]
                if ssl[1] - ssl[0] < nd:
                    src = src.broadcast_to([P, H, nd])
                nc.vector.tensor_tensor(t3[:, :, :nd], dst, src, AL.add)
                nc.vector.tensor_scalar_mul(dst, t3[:, :, :nd], 0.5)
    else:
        dummy = small.tile([P, 1], f32, tag="dummy")
        for dst, src in CHAIN:
            # e[dst] = (e[dst] + e[src]) * 0.5 fused via TTR (accum is a dummy)
            nc.vector.tensor_tensor_reduce(
                eav[:, :, 0, dst], eav[:, :, 0, dst], eav[:, :, 0, src],
                0.5, 0.0, AL.add, AL.max, dummy[:])
        for dst, src in CHAIN:
            nc.vector.tensor_tensor_reduce(
                uav[:, :, dst], uav[:, :, dst], uav[:, :, src],
                0.5, 0.0, AL.add, AL.max, dummy[:])
    du_all = small.tile([P, H * QJ], f32, tag="du_all")
    nc.vector.tensor_tensor(du_all[:], u_all[:], u_orig[:], AL.subtract)
    a0_all = work.tile([P, H * QJ * D], f32, tag="a0_all")
    for h in range(H):
        p2 = work.tile([P, QJ * D * KJ], bf, tag="p2")
        p2v = p2[:].rearrange("p (q d k) -> p q d k", q=QJ, d=D)
        e_b = eav[:, h].unsqueeze(2).broadcast_to([P, QJ, D, KJ])
        va = va_all[:, h * NQK:(h + 1) * NQK]
        va_b = va.rearrange("p (d k) -> p d k", d=D).unsqueeze(1).broadcast_to([P, QJ, D, KJ])
        nc.vector.tensor_tensor(p2v, e_b, va_b, AL.mult)
        if V1_KJSUM:
            nc.vector.tensor_reduce(
                a0_all[:, h * NQK:(h + 1) * NQK].rearrange("p (q d) -> p q d", q=QJ),
                p2v, axis=AX.X, op=AL.add)
        else:
            # kj-sum via 2x-friendly tree (4B-aligned slice pairs): 24->12->6->2->1
            p2f = p2[:].rearrange("p (f k) -> p f k", f=NQK)
            t12 = work.tile([P, NQK * 12], bf, tag="t12")
            t12v = t12[:].rearrange("p (f k) -> p f k", f=NQK)
            nc.vector.tensor_tensor(t12v, p2f[:, :, 0:12], p2f[:, :, 12:24], AL.add)
            t6 = work.tile([P, NQK * 6], bf, tag="t6")
            t6v = t6[:].rearrange("p (f k) -> p f k", f=NQK)
            nc.vector.tensor_tensor(t6v, t12v[:, :, 0:6], t12v[:, :, 6:12], AL.add)
            t2 = work.tile([P, NQK * 2], bf, tag="t2")
            t2v = t2[:].rearrange("p (f k) -> p f k", f=NQK)
            nc.vector.tensor_tensor(t2v, t6v[:, :, 0:2], t6v[:, :, 2:4], AL.add)
            nc.vector.tensor_tensor(t2v, t2v, t6v[:, :, 4:6], AL.add)
            nc.vector.tensor_tensor(
                a0_all[:, h * NQK:(h + 1) * NQK], t2v[:, :, 0], t2v[:, :, 1], AL.add)
    a0v = a0_all[:].rearrange("p (h q d) -> p h q d", h=H, q=QJ)
    rb = rav.unsqueeze(3).broadcast_to([P, H, QJ, D])
    corr = work.tile([P, H * QJ * D], f32, tag="corr")
    corrv = corr[:].rearrange("p (h q d) -> p h q d", h=H, q=QJ)
    du_b = du_all[:].rearrange("p (h q) -> p h q", h=H).unsqueeze(3).broadcast_to([P, H, QJ, D])
    va0 = va_all[:].rearrange("p (h d k) -> p h d k", h=H, d=D)[:, :, :, 0]
    va0_b = va0.unsqueeze(2).broadcast_to([P, H, QJ, D])
    nc.vector.tensor_tensor(corrv, du_b, va0_b, AL.mult)
    nc.vector.tensor_tensor(a0v, a0v, rb, AL.mult)
    att_dst = vptok[:].rearrange("p (h c) -> p h c", h=H)[:, :, 3:]
    att_dst = att_dst.rearrange("p h (q d) -> p h q d", q=QJ)
    nc.vector.tensor_tensor(att_dst, a0v, corrv, AL.add)


def build_program(tt=TT, inner_repeat=1):
    nc = bacc.Bacc("TRN2", target_bir_lowering=False, debug=False)
    T = tt * P
    q_dram = nc.dram_tensor("query", [T, D_MODEL], f32, kind="ExternalInput").ap()
    v_dram = nc.dram_tensor("value", [T, D_MODEL], f32, kind="ExternalInput").ap()
    wcat_dram = nc.dram_tensor("w_cat", [D_MODEL, W_TOT], bf, kind="ExternalInput").ap()
    lw_dram = nc.dram_tensor("lin_w", [DH, D_MODEL], bf, kind="ExternalInput").ap()
    lb_dram = nc.dram_tensor("lin_b", [1, D_MODEL], f32, kind="ExternalInput").ap()
    out_dram = nc.dram_tensor("out", [T, D_MODEL], f32, kind="ExternalOutput").ap()

    with tile.TileContext(nc) as tc, ExitStack() as ctx:
        const = ctx.enter_context(tc.tile_pool(name="const", bufs=1))
        wpool = ctx.enter_context(tc.tile_pool(name="wpool", bufs=1))
        io = ctx.enter_context(tc.tile_pool(name="io", bufs=2))
        xt = ctx.enter_context(tc.tile_pool(name="xt", bufs=2))
        qkv = ctx.enter_context(tc.tile_pool(name="qkv", bufs=2))
        work = ctx.enter_context(tc.tile_pool(name="work", bufs=int(_os.environ.get("WORK_BUFS") or 2)))
        small = ctx.enter_context(tc.tile_pool(name="small", bufs=2))
        vt = ctx.enter_context(tc.tile_pool(name="vt", bufs=2))
        outp = ctx.enter_context(tc.tile_pool(name="outp", bufs=2))
        ps_t = ctx.enter_context(tc.tile_pool(name="ps_t", bufs=int(_os.environ.get("PST_BUFS") or 2), space="PSUM"))
        ps_mm = ctx.enter_context(tc.tile_pool(name="ps_mm", bufs=int(_os.environ.get("PSMM_BUFS") or 2), space="PSUM"))
        ps_out = ctx.enter_context(tc.tile_pool(name="ps_out", bufs=2, space="PSUM"))

        ident = const.tile([P, P], bf, tag="ident")
        make_identity(nc, ident[:])
        bias_bc = const.tile([P, D_MODEL], f32, tag="bias_bc")
        bias_row = const.tile([1, D_MODEL], f32, tag="bias_row")
        nc.sync.dma_start(bias_row[:], lb_dram[:])
        nc.gpsimd.partition_broadcast(bias_bc[:], bias_row[:])
        wcat = []
        for k in range(4):
            wk = wpool.tile([P, W_TOT], bf, tag=f"wcat{k}")
            nc.sync.dma_start(wk[:], wcat_dram[k * P:(k + 1) * P, :])
            wcat.append(wk)
        lw = []
        for k in range(10):
            rows = min(P, DH - k * P)
            lwk = wpool.tile([P, D_MODEL], bf, tag=f"lw{k}")
            nc.sync.dma_start(lwk[:rows, :], lw_dram[k * P:k * P + rows, :])
            lw.append((lwk, rows))

        for it in range(tt):
          for _rep in range(inner_repeat):
            q_f32 = io.tile([P, D_MODEL], f32, tag="q_f32")
            v_f32 = io.tile([P, D_MODEL], f32, tag="v_f32")
            nc.sync.dma_start(q_f32[:], q_dram[it * P:(it + 1) * P, :])
            nc.sync.dma_start(v_f32[:], v_dram[it * P:(it + 1) * P, :])
            q_bf = io.tile([P, D_MODEL], bf, tag="q_bf")
            v_bf = io.tile([P, D_MODEL], bf, tag="v_bf")
            nc.scalar.copy(q_bf[:], q_f32[:])
            nc.scalar.copy(v_bf[:], v_f32[:])
            xqT, xvT = [], []
            for src, dstlist, nm in ((q_bf, xqT, "q"), (v_bf, xvT, "v")):
                for k in range(4):
                    pst = ps_t.tile([P, P], bf, tag="pst")
                    nc.tensor.transpose(pst[:], src[:, k * P:(k + 1) * P], ident[:])
                    xk = xt.tile([P, P], bf, tag=f"x{nm}T{k}")
                    nc.scalar.copy(xk[:], pst[:])
                    dstlist.append(xk)
            qa_all = qkv.tile([P, NVA], bf, tag="qa_all")
            ka_all = qkv.tile([P, NVA], bf, tag="ka_all")
            va_all = qkv.tile([P, NVA], bf, tag="va_all")
            vptok = qkv.tile([P, DH], bf, tag="vptok")
            for (c0, c1, kind) in QKV_CHUNKS:
                w_n = c1 - c0
                pmm = ps_mm.tile([P, 512], f32, tag="pmm")
                lhs_tiles = xvT if kind == 2 else xqT
                for k in range(4):
                    nc.tensor.matmul(pmm[:, :w_n], lhs_tiles[k][:], wcat[k][:, c0:c1],
                                     start=(k == 0), stop=(k == 3))
                if kind == 0:
                    nc.scalar.copy(qa_all[:, c0:c1], pmm[:, :w_n])
                elif kind == 1:
                    nc.scalar.copy(ka_all[:, c0 - NVA:c1 - NVA], pmm[:, :w_n])
                else:
                    v0, v1 = c0 - 2 * NVA, c1 - 2 * NVA
                    if v1 <= NVA:
                        nc.scalar.copy(va_all[:, v0:v1], pmm[:, :w_n])
                    else:
                        nc.scalar.copy(va_all[:, v0:NVA], pmm[:, :NVA - v0])
                        vp = pmm[:, NVA - v0:w_n].rearrange("p (h c) -> p h c", h=H)
                        vp_dst = vptok[:].rearrange("p (h c) -> p h c", h=H)[:, :, :3]
                        nc.scalar.copy(vp_dst, vp)
            _inner_attention(tc, work, small, qa_all, ka_all, va_all, vptok)
            vT = []
            for k in range(10):
                cols = min(P, DH - k * P)
                pst2 = ps_t.tile([P, P], bf, tag="pst2")
                nc.tensor.transpose(pst2[:cols, :], vptok[:, k * P:k * P + cols], ident[:])
                vk = vt.tile([P, P], bf, tag=f"vT{k}")
                nc.scalar.copy(vk[:cols, :], pst2[:cols, :])
                vT.append((vk, cols))
            pout = ps_out.tile([P, D_MODEL], f32, tag="pout")
            for k in range(10):
                vk, rows = vT[k]
                lwk, rows2 = lw[k]
                nc.tensor.matmul(pout[:], vk[:rows, :], lwk[:rows, :],
                                 start=(k == 0), stop=(k == 9))
            out_sb = outp.tile([P, D_MODEL], f32, tag="out_sb")
            nc.vector.tensor_tensor(out_sb[:], pout[:], bias_bc[:], AL.add)
            nc.sync.dma_start(out_dram[it * P:(it + 1) * P, :], out_sb[:])

    nc.compile()
    return nc


def prep_weights(qk_w, v_w, lin_w, lin_b):
    bfl = ml_dtypes.bfloat16 if _os.environ.get('LP_BF16') else np.float16
    scale = np.float32(1.0 / np.sqrt(6.0))
    wq = np.asarray(qk_w[:, :DH], dtype=np.float32).reshape(D_MODEL, H, 147)
    wk = np.asarray(qk_w[:, DH:], dtype=np.float32).reshape(D_MODEL, H, 147)
    wv = np.asarray(v_w, dtype=np.float32).reshape(D_MODEL, H, 147)
    wq_p = (wq[:, :, 3:] * scale).reshape(D_MODEL, H * NQK)
    wk_p = wk[:, :, 3:].reshape(D_MODEL, H * NQK)
    wv_att = wv[:, :, 3:].reshape(D_MODEL, H, KJ, D).transpose(0, 1, 3, 2).reshape(D_MODEL, H * NQK)
    wv_pass = wv[:, :, :3].reshape(D_MODEL, H * 3)
    w_cat = np.ascontiguousarray(
        np.concatenate([wq_p, wk_p, wv_att, wv_pass], axis=1)).astype(bfl)
    lw_bf = np.ascontiguousarray(np.asarray(lin_w, dtype=np.float32)).astype(bfl)
    lb = np.ascontiguousarray(np.asarray(lin_b, dtype=np.float32).reshape(1, D_MODEL))
    return w_cat, lw_bf, lb


_CACHED_NC = None


def _get_nc():
    global _CACHED_NC
    if _CACHED_NC is None:
        _CACHED_NC = build_program(TT)
    return _CACHED_NC


def kernel(query, key, value, qk_w, v_w, lin_w, lin_b, _want_results=False, **_ignored):
    """Full-input kernel: shards batch over 8 cores, returns full output."""
    query = np.asarray(query, dtype=np.float32)
    value = np.asarray(value, dtype=np.float32)
    w_cat, lw_bf, lb = prep_weights(qk_w, v_w, lin_w, lin_b)
    nc = _get_nc()
    bpc = B // N_CORES
    in_maps = []
    for c in range(N_CORES):
        in_maps.append({
            "query": np.ascontiguousarray(query[c * bpc:(c + 1) * bpc].reshape(-1, D_MODEL)),
            "value": np.ascontiguousarray(value[c * bpc:(c + 1) * bpc].reshape(-1, D_MODEL)),
            "w_cat": w_cat,
            "lin_w": lw_bf,
            "lin_b": lb,
        })
    res = run_bass_kernel_spmd(nc, in_maps, core_ids=list(range(N_CORES)))
    out = np.empty((B, N, D_MODEL), dtype=np.float32)
    for c in range(N_CORES):
        out[c * bpc:(c + 1) * bpc] = res.results[c]["out"].reshape(bpc, N, D_MODEL)
    if _want_results:
        return out, res
    return out

